# revision 42
# baseline (speedup 1.0000x reference)
"""2-layer GAT (graph attention) Bass/Tile kernel for Trainium2, 8-core SPMD.

Sharding: nodes partitioned contiguously across cores (rank-major ids
id = r*Nlp + local); edges assigned to the core owning their dst, sorted by
dst, grouped into 128-dst blocks with uniform slot padding so all cores share
one SPMD module.

Per core: build feature-table rows [feat bf16 | el bf16 | pad] for the LOCAL
nodes only (one matmul per 128-node block against a host-prebuilt rhs that
also produces the attention el/er columns), then AllGather the table so every
core holds all N rows.  Edge phase per 128-dst block: two dma_gathers (lo/hi
int16 index halves) fetch src rows; one-hot dst masks are built in a single
DVE op per block and drive PE matmuls that broadcast er to edges and
scatter-add the exp-weighted messages + exp-sums into PSUM (softmax
max-subtraction is skipped; |e| is O(1)).  The layer-2 table row (h@W2 and
its el/er columns) is computed inline right after layer-1 normalization+ELU,
so hidden states never round-trip HBM; a second AllGather publishes tab2.
er columns for local dst nodes stay resident in SBUF.  Outputs accumulate in
SBUF and are written once.
"""

import os

import numpy as np

import concourse.bacc as bacc
import concourse.bass as bass
import concourse.mybir as mybir
import concourse.tile as tile
from concourse.masks import make_identity

F32 = mybir.dt.float32
BF16 = mybir.dt.bfloat16
I32 = mybir.dt.int32
I16 = mybir.dt.int16
I8 = mybir.dt.int8
AF = mybir.ActivationFunctionType
OP = mybir.AluOpType

P = 128
NPBF = None  # numpy bfloat16 dtype, resolved lazily


def _npbf():
    global NPBF
    if NPBF is None:
        NPBF = mybir.dt.np(BF16)
    return NPBF


class GATCfg:
    def __init__(self, N=50000, C=8, IN=128, HID=32, HEADS=8, OUT=16, NEG=0.2):
        self.N, self.C, self.IN = N, C, IN
        self.HID, self.HEADS, self.OUT, self.NEG = HID, HEADS, OUT, NEG
        self.F1 = HEADS * HID
        self.F2 = HEADS * OUT
        self.Nloc = (N + C - 1) // C
        self.NB = (self.Nloc + P - 1) // P
        self.Nlp = self.NB * P
        self.NP2 = C * self.Nlp            # rank-major padded id space
        # int16 gather index split: lo ids < HALF, hi ids in [HALF, NP2)
        h = (self.NP2 // 2 + P - 1) // P * P
        self.HALF = min(32768, max(h, self.NP2 - 32768))
        assert self.HALF <= 32768 and self.NP2 - self.HALF <= 32768
        self.SLO = 0
        self.SHI = 0
        # table rows (bf16 elems), 256B-multiple for dma_gather
        self.ROW1 = ((self.F1 + 8 + 127) // 128) * 128   # 384 -> 768B
        self.ROW2 = ((self.F2 + 8 + 127) // 128) * 128   # 256 -> 512B
        self.GRP = 6   # metadata load group (even: gathers pair 2 blocks)

    @property
    def NCt(self):
        return (self.SLO + self.SHI) // P


def _wrap16(vals_slots, NB, S):
    """[NB, S] slot-ordered ints -> [128, NB*S//16] 16-wrapped, replicated 8x."""
    a = vals_slots.reshape(NB, S // 16, 16)
    out = np.zeros((128, NB * (S // 16)), np.int16)
    for b in range(NB):
        blkcols = a[b].T.astype(np.int16)          # [16, S//16]
        for r in range(8):
            out[r * 16:(r + 1) * 16,
                b * (S // 16):(b + 1) * (S // 16)] = blkcols
    return out


def prep_indices(src, dst, cfg):
    """Host index-only preprocessing (single shared id space, rank-major)."""
    C, Nloc, NB = cfg.C, cfg.Nloc, cfg.NB
    src = np.asarray(src).astype(np.int64)
    dst = np.asarray(dst).astype(np.int64)
    core = dst // Nloc
    dloc = dst - core * Nloc
    blk = dloc // P
    dblk = dloc - blk * P
    key = core * NB + blk
    order = np.argsort(key, kind="stable")
    counts = np.bincount(key, minlength=C * NB)
    starts = np.zeros(C * NB + 1, np.int64)
    np.cumsum(counts, out=starts[1:])

    r_of = src // Nloc
    ids = r_of * cfg.Nlp + (src - r_of * Nloc)     # rank-major id
    HALFc = cfg.HALF

    lo_max = hi_max = 0
    for k in range(C * NB):
        e = order[starts[k]:starts[k + 1]]
        n_lo = int((ids[e] < HALFc).sum())
        lo_max = max(lo_max, n_lo)
        hi_max = max(hi_max, e.size - n_lo)
    SLO = ((max(lo_max, 1) + P - 1) // P) * P
    SHI = ((max(hi_max, 1) + P - 1) // P) * P
    cfg.SLO, cfg.SHI = SLO, SHI
    S = SLO + SHI
    NCt = S // P

    idxlo = np.zeros((C, NB, SLO), np.int64)
    idxhi = np.zeros((C, NB, max(SHI, 16)), np.int64)
    dstb = np.full((C, P, NB * NCt), -1.0, np.float32)
    dbf = np.full((C, NB * NCt * P), -1, np.int8)
    for c in range(C):
        for b in range(NB):
            k = c * NB + b
            e = order[starts[k]:starts[k + 1]]
            v = ids[e]
            m = v < HALFc
            elo, ehi = e[m], e[~m]
            idxlo[c, b, :elo.size] = v[m]
            if ehi.size:
                idxhi[c, b, :ehi.size] = v[~m] - HALFc
            sl = np.concatenate([np.arange(elo.size),
                                 SLO + np.arange(ehi.size)])
            ee = np.concatenate([elo, ehi])
            dstb[c, sl % P, b * NCt + sl // P] = dblk[ee]
            dbf[c, b * NCt * P + sl] = dblk[ee]
    wlo = np.stack([_wrap16(idxlo[c], NB, SLO) for c in range(C)])
    whi = np.stack([_wrap16(idxhi[c], NB, max(SHI, 16)) for c in range(C)])
    return {"idxlo": wlo, "idxhi": whi, "dstb": dstb, "dbb": dbf}


def _alr_mat(al, ar, F, H, D):
    m = np.zeros((F, 16), np.float32)
    for h in range(H):
        m[h * D:(h + 1) * D, h] = al[h]
        m[h * D:(h + 1) * D, 8 + h] = ar[h]
    return m


def _dmaj(a, heads):
    """Permute the last axis from head-major (h*D+d) to d-major (d*H+h)."""
    D = a.shape[-1] // heads
    return np.ascontiguousarray(
        a.reshape(*a.shape[:-1], heads, D).swapaxes(-1, -2)
        .reshape(*a.shape[:-1], heads * D))


def host_inputs(inputs, cfg, idx):
    nbf = _npbf()
    H = cfg.HEADS
    x = np.asarray(inputs["x"], np.float32)
    W1 = np.asarray(inputs["W1"], np.float32)
    W2 = np.asarray(inputs["W2"], np.float32)
    alr1 = _alr_mat(np.asarray(inputs["al1"], np.float32),
                    np.asarray(inputs["ar1"], np.float32),
                    cfg.F1, cfg.HEADS, cfg.HID)
    alr2 = _alr_mat(np.asarray(inputs["al2"], np.float32),
                    np.asarray(inputs["ar2"], np.float32),
                    cfg.F2, cfg.HEADS, cfg.OUT)
    # feature columns stored d-major (d*H+h) so the per-head exp broadcast in
    # the edge phase has a packed innermost dim (2x DVE mode); el/er columns
    # stay head-indexed.
    rhs1 = np.concatenate([_dmaj(W1, H), W1 @ alr1], axis=1).astype(nbf)
    rhs2 = np.concatenate([_dmaj(W2, H), W2 @ alr2], axis=1)       # [256,144]
    # rows of rhs2 live in feat1 space -> permute to d-major to match hb
    rhs2 = np.ascontiguousarray(
        rhs2.reshape(H, cfg.HID, cfg.F2 + 16).swapaxes(0, 1)
        .reshape(cfg.F1, cfg.F2 + 16)).astype(nbf)
    b1row = np.broadcast_to(_dmaj(np.asarray(inputs["b1"], np.float32)
                                  .reshape(1, cfg.F1), H), (P, cfg.F1)).copy()
    b2m = np.asarray(inputs["b2"], np.float32).reshape(
        cfg.HEADS, cfg.OUT).mean(axis=0)
    b2mean = np.broadcast_to(b2m.reshape(1, cfg.OUT), (P, cfg.OUT)).copy()

    xT = np.ascontiguousarray(x.T)                                  # [IN, N]
    in_maps = []
    for c in range(cfg.C):
        xTl = np.zeros((cfg.IN, cfg.Nlp), nbf)
        lo = c * cfg.Nloc
        hi = min(cfg.N, lo + cfg.Nloc)
        xTl[:, :hi - lo] = xT[:, lo:hi].astype(nbf)
        in_maps.append({
            "xTl": xTl, "RHS1": rhs1, "RHS2": rhs2,
            "B1R": b1row, "B2M": b2mean,
            "idxlo": idx["idxlo"][c], "idxhi": idx["idxhi"][c],
            "dstb": idx["dstb"][c].astype(nbf),
            "dbb": np.ascontiguousarray(
                np.broadcast_to(idx["dbb"][c][None, :],
                                (P, idx["dbb"].shape[1]))),
        })
    return in_maps


def build_module(cfg, dbg=False, skip_cc=False):
    nc = bacc.Bacc("TRN2", target_bir_lowering=False, debug=False,
                   num_devices=cfg.C, dynamic_dma_scratch_size=24576)
    C, NB, Nlp = cfg.C, cfg.NB, cfg.Nlp
    F1, F2, ROW1, ROW2 = cfg.F1, cfg.F2, cfg.ROW1, cfg.ROW2
    SLO, SHI, NCt = cfg.SLO, cfg.SHI, cfg.NCt
    NLOC, NHIC = SLO // P, SHI // P
    G = cfg.GRP
    NG = (NB + G - 1) // G
    CL = SLO // 16
    CH = max(SHI, 16) // 16

    d_xTl = nc.dram_tensor("xTl", [cfg.IN, Nlp], BF16, kind="ExternalInput")
    d_rhs1 = nc.dram_tensor("RHS1", [cfg.IN, F1 + 16], BF16,
                            kind="ExternalInput")
    d_rhs2 = nc.dram_tensor("RHS2", [F1, F2 + 16], BF16,
                            kind="ExternalInput")
    d_b1r = nc.dram_tensor("B1R", [P, F1], F32, kind="ExternalInput")
    d_b2m = nc.dram_tensor("B2M", [P, cfg.OUT], F32, kind="ExternalInput")
    d_il = nc.dram_tensor("idxlo", [P, NB * CL], I16, kind="ExternalInput")
    d_ih = nc.dram_tensor("idxhi", [P, NB * CH], I16, kind="ExternalInput")
    d_db = nc.dram_tensor("dstb", [P, NB * NCt], BF16, kind="ExternalInput")
    d_dbb = nc.dram_tensor("dbb", [P, NB * NCt * P], I8, kind="ExternalInput")
    d_out = nc.dram_tensor("out", [cfg.Nloc, cfg.OUT], F32,
                           kind="ExternalOutput")

    # NOTE: dma_gather cannot read Shared-space tensors (neuronxcc
    # DataLocalityOpt asserts), so the AllGather outputs stay Local.
    shared = os.environ.get("GAT_CC_SPACE", "Local")
    d_tab1l = nc.dram_tensor("tab1l", [Nlp, ROW1], BF16, kind="Internal")
    d_tab2l = nc.dram_tensor("tab2l", [Nlp, ROW2], BF16, kind="Internal")
    d_tab1g = nc.dram_tensor("tab1g", [cfg.NP2, ROW1], BF16, kind="Internal",
                             addr_space=shared)
    d_tab2g = nc.dram_tensor("tab2g", [cfg.NP2, ROW2], BF16, kind="Internal",
                             addr_space=shared)

    with tile.TileContext(nc) as tc:
        with (
            tc.tile_pool(name="const", bufs=1) as cpool,
            tc.tile_pool(name="meta", bufs=2) as mpool,
        ):
            # ---------------- constants ----------------
            iota_i = cpool.tile([P, P], I32)
            nc.gpsimd.iota(iota_i[:], pattern=[[1, P]], base=0,
                           channel_multiplier=0)
            iota_p = cpool.tile([P, P], BF16)
            nc.vector.tensor_copy(iota_p[:], iota_i[:])
            # [P, j, c] iota (value j, constant along c): the oh build's db
            # broadcast then has a packed innermost dim -> 2x DVE mode
            iotaJc = cpool.tile([P, P, NCt], BF16)
            nc.vector.tensor_copy(
                iotaJc[:],
                iota_p[:].rearrange("p (j one) -> p j one", one=1)
                .to_broadcast([P, P, NCt]))
            iota_ci = cpool.tile([P, 1], I32)
            nc.gpsimd.iota(iota_ci[:], pattern=[[1, 1]], base=0,
                           channel_multiplier=1)
            iota_cf = cpool.tile([P, 1], F32)
            nc.vector.tensor_copy(iota_cf[:], iota_ci[:])
            ident = cpool.tile([P, P], BF16)
            make_identity(nc, ident[:])

            rhs1 = cpool.tile([P, F1 + 16], BF16)
            nc.sync.dma_start(rhs1[:], d_rhs1[:, :])
            rhs2 = cpool.tile([P, 2, F2 + 16], BF16)
            nc.sync.dma_start(rhs2[:, 0, :], d_rhs2[0:P, :])
            nc.sync.dma_start(rhs2[:, 1, :], d_rhs2[P:F1, :])
            b1row = cpool.tile([P, F1], F32)
            nc.sync.dma_start(b1row[:], d_b1r[:, :])
            b2mean = cpool.tile([P, cfg.OUT], F32)
            nc.sync.dma_start(b2mean[:], d_b2m[:, :])

            er1_sb = cpool.tile([P, NB, 8], BF16)
            er2_sb = cpool.tile([P, NB, 8], BF16)
            tab2acc = cpool.tile([P, NB, F2 + 8], BF16)
            outacc = cpool.tile([P, NB, cfg.OUT], F32)

            # ---------------- layer-1 local table ----------------
            with tc.tile_pool(name="p1", bufs=1) as p1pool, \
                 tc.tile_pool(name="t1ps", bufs=3, space="PSUM") as t1ps:
                xTl_sb = p1pool.tile([P, Nlp], BF16)
                nc.sync.dma_start(xTl_sb[:], d_xTl[:, :])
                tab1acc = p1pool.tile([P, NB, F1 + 8], BF16)
                for t in range(NB):
                    ps = t1ps.tile([P, F1 + 16], F32, tag="tbl")
                    nc.tensor.matmul(ps[:], lhsT=xTl_sb[:, t * P:(t + 1) * P],
                                     rhs=rhs1[:], start=True, stop=True)
                    nc.scalar.copy(tab1acc[:, t, :], ps[:, 0:F1 + 8])
                    nc.vector.tensor_copy(er1_sb[:, t, :],
                                          ps[:, F1 + 8:F1 + 16])
                nc.sync.dma_start(
                    d_tab1l[:, 0:F1 + 8].rearrange("(t p) r -> p t r", p=P),
                    tab1acc[:])

            # ---------------- allgather tab1 ----------------
            if C > 1 and not skip_cc:
                nc.gpsimd.collective_compute(
                    "AllGather", OP.bypass,
                    replica_groups=[list(range(C))],
                    ins=[d_tab1l[:, :]],
                    outs=[d_tab1g[:, :]],
                )

            # ---------------- edge phase (shared) ----------------
            def edge_phase(layer, wpool, gpool, pspool, tps):
                F = F1 if layer == 1 else F2
                ROW = ROW1 if layer == 1 else ROW2
                if C > 1:
                    tab = d_tab1g if layer == 1 else d_tab2g
                else:
                    tab = d_tab1l if layer == 1 else d_tab2l
                er_sb = er1_sb if layer == 1 else er2_sb
                ablate = os.environ.get("GAT_ABLATE", "")
                pair = None
                for b in range(NB):
                    g = b // G
                    j = b % G
                    gn = min(G, NB - g * G)
                    if j == 0:
                        il_g = mpool.tile([P, G * CL], I16, tag="il")
                        nc.sync.dma_start(
                            il_g[:, 0:gn * CL],
                            d_il[:, g * G * CL:(g * G + gn) * CL])
                        ih_g = mpool.tile([P, G * CH], I16, tag="ih")
                        nc.sync.dma_start(
                            ih_g[:, 0:gn * CH],
                            d_ih[:, g * G * CH:(g * G + gn) * CH])
                        db_g = mpool.tile([P, G * NCt], BF16, tag="db")
                        nc.sync.dma_start(
                            db_g[:, 0:gn * NCt],
                            d_db[:, g * G * NCt:(g * G + gn) * NCt])
                        dbb_g = mpool.tile([P, G * NCt * P], I8, tag="dbb")
                        nc.sync.dma_start(
                            dbb_g[:, 0:gn * NCt * P],
                            d_dbb[:, g * G * NCt * P:(g * G + gn) * NCt * P])
                    PIECE = 768  # max idxs per dma_gather (packet cap)
                    if b % 2 == 0:
                        # gather a PAIR of blocks per tile so 768-idx pieces
                        # don't fragment at block boundaries
                        pn = min(2, NB - b)
                        G2lo = gpool.tile([P, 2 * NLOC, ROW], BF16,
                                          tag=f"Glo{layer}")
                        for s0 in ([] if "nogather" in ablate
                                   else range(0, pn * SLO, PIECE)):
                            n = min(PIECE, pn * SLO - s0)
                            nc.gpsimd.dma_gather(
                                out_ap=G2lo[:, s0 // P:(s0 + n) // P, :],
                                in_ap=tab[:, :],
                                idxs_ap=il_g[:, j * CL + s0 // 16:
                                             j * CL + (s0 + n) // 16],
                                num_idxs=n, num_idxs_reg=n, elem_size=ROW)
                        G2hi = gpool.tile([P, 2 * NHIC, ROW], BF16,
                                          tag=f"Ghi{layer}")
                        for s0 in ([] if "nogather" in ablate
                                   else range(0, pn * SHI, PIECE)):
                            n = min(PIECE, pn * SHI - s0)
                            nc.gpsimd.dma_gather(
                                out_ap=G2hi[:, s0 // P:(s0 + n) // P, :],
                                in_ap=tab[cfg.HALF:, :],
                                idxs_ap=ih_g[:, j * CH + s0 // 16:
                                             j * CH + (s0 + n) // 16],
                                num_idxs=n, num_idxs_reg=n, elem_size=ROW)
                        pair = (G2lo, G2hi)
                    G2lo, G2hi = pair
                    ho = (b % 2) * NLOC
                    hh = (b % 2) * NHIC

                    # one-hot masks, one DVE op each for the whole block
                    ohT = wpool.tile([P, NCt, P], BF16, tag="ohT")
                    nc.vector.tensor_scalar(
                        ohT[:], dbb_g[:, j * NCt * P:(j + 1) * NCt * P]
                        .rearrange("p (c j) -> p c j", c=NCt),
                        iota_cf[:, 0:1], None, op0=OP.is_equal)
                    oh = wpool.tile([P, P, NCt], BF16, tag="oh")
                    nc.vector.tensor_tensor(
                        out=oh[:], in0=iotaJc[:],
                        in1=db_g[:, j * NCt:(j + 1) * NCt]
                        .rearrange("p (one c) -> p one c", one=1)
                        .to_broadcast([P, P, NCt]),
                        op=OP.is_equal)

                    # er broadcast to edges: per-chunk PE matmul
                    erps = tps.tile([P, NCt, 8], F32, tag="erps")
                    for c in range(NCt):
                        nc.tensor.matmul(erps[:, c, :], lhsT=ohT[:, c, :],
                                         rhs=er_sb[:, b, :],
                                         start=True, stop=True)
                    e_all = wpool.tile([P, NCt, 8], F32, tag="e_all")
                    nc.vector.tensor_tensor(
                        out=e_all[:, 0:NLOC, :],
                        in0=G2lo[:, ho:ho + NLOC, F:F + 8],
                        in1=erps[:, 0:NLOC, :], op=OP.add)
                    nc.vector.tensor_tensor(
                        out=e_all[:, NLOC:NCt, :],
                        in0=G2hi[:, hh:hh + NHIC, F:F + 8],
                        in1=erps[:, NLOC:NCt, :], op=OP.add)
                    # leaky relu in one fused op: max(e, 0.2*e)
                    es_a = wpool.tile([P, NCt, 8], F32, tag="es_a")
                    nc.vector.scalar_tensor_tensor(
                        es_a[:], e_all[:], cfg.NEG, e_all[:],
                        op0=OP.mult, op1=OP.max)
                    MSG = gpool.tile([P, NCt, F + 8], BF16, tag=f"MSG{layer}")
                    nc.scalar.activation(MSG[:, :, F:F + 8], es_a[:], AF.Exp)

                    # feat cols are d-major: per-head exp broadcast has a
                    # packed innermost dim (2x DVE mode)
                    def _mult(mout, gin, exin, nchunk):
                        nc.vector.tensor_tensor(
                            out=mout.rearrange("p c (d h) -> p c d h", h=8),
                            in0=gin.rearrange("p c (d h) -> p c d h", h=8),
                            in1=exin.rearrange("p c (one h) -> p c one h",
                                               one=1)
                                .to_broadcast([P, nchunk, F // 8, 8]),
                            op=OP.mult)

                    _mult(MSG[:, 0:NLOC, 0:F], G2lo[:, ho:ho + NLOC, 0:F],
                          MSG[:, 0:NLOC, F:F + 8], NLOC)
                    _mult(MSG[:, NLOC:NCt, 0:F], G2hi[:, hh:hh + NHIC, 0:F],
                          MSG[:, NLOC:NCt, F:F + 8], NHIC)
                    ps = pspool.tile([P, F + 8], F32, tag="eps")
                    for c in range(NCt):
                        nc.tensor.matmul(ps[:], lhsT=oh[:, :, c],
                                         rhs=MSG[:, c, :],
                                         start=(c == 0), stop=(c == NCt - 1))

                    esum = wpool.tile([P, 8], F32, tag="esum")
                    nc.vector.tensor_scalar(esum[:], ps[:, F:F + 8], 1e-30,
                                            None, op0=OP.max)
                    inv = wpool.tile([P, 8], F32, tag="inv")
                    nc.vector.reciprocal(inv[:], esum[:])
                    yield b, ps, inv

            # ---------------- layer-1 edges + fused tab2 rows ----------------
            with tc.tile_pool(name="w1", bufs=2) as wpool, \
                 tc.tile_pool(name="g1", bufs=2) as gpool, \
                 tc.tile_pool(name="e1ps", bufs=2, space="PSUM") as e1ps, \
                 tc.tile_pool(name="tps", bufs=2, space="PSUM") as tps, \
                 tc.tile_pool(name="t2ps", bufs=2, space="PSUM") as t2ps:
                for b, ps, inv in edge_phase(1, wpool, gpool, e1ps, tps):
                    z = wpool.tile([P, F1], F32, tag="z")
                    nc.vector.tensor_tensor(
                        out=z[:].rearrange("p (d h) -> p d h", h=8),
                        in0=ps[:, 0:F1].rearrange("p (d h) -> p d h", h=8),
                        in1=inv[:].rearrange("p (one h) -> p one h", one=1)
                            .to_broadcast([P, cfg.HID, 8]),
                        op=OP.mult)
                    nc.vector.tensor_add(z[:], z[:], b1row[:])
                    # elu(z) = relu(z) + (exp(min(z,0)) - 1)
                    zp = wpool.tile([P, F1], F32, tag="zp")
                    nc.scalar.activation(zp[:], z[:], AF.Relu)
                    zm = wpool.tile([P, F1], F32, tag="zm")
                    nc.vector.tensor_sub(zm[:], z[:], zp[:])
                    q_ = wpool.tile([P, F1], F32, tag="q_")
                    nc.scalar.activation(q_[:], zm[:], AF.Exp)
                    hb = wpool.tile([P, F1], BF16, tag="hb")
                    nc.vector.scalar_tensor_tensor(
                        hb[:], q_[:], -1.0, zp[:], op0=OP.add, op1=OP.add)
                    # fused layer-2 table row: transpose h, matmul rhs2
                    ps2 = t2ps.tile([P, F2 + 16], F32, tag="ps2")
                    for q in range(2):
                        pst = t2ps.tile([P, P], BF16, tag="pst")
                        nc.tensor.transpose(pst[:], hb[:, q * P:(q + 1) * P],
                                            ident[:])
                        htp = wpool.tile([P, P], BF16, tag=f"htp{q}")
                        (nc.vector.tensor_copy if q == 0
                         else nc.scalar.copy)(htp[:], pst[:])
                        nc.tensor.matmul(ps2[:], lhsT=htp[:],
                                         rhs=rhs2[:, q, :],
                                         start=(q == 0), stop=(q == 1))
                    nc.scalar.copy(tab2acc[:, b, :], ps2[:, 0:F2 + 8])
                    nc.scalar.copy(er2_sb[:, b, :], ps2[:, F2 + 8:F2 + 16])
                nc.sync.dma_start(
                    d_tab2l[:, 0:F2 + 8].rearrange("(t p) r -> p t r", p=P),
                    tab2acc[:])

            # ---------------- allgather tab2 ----------------
            if C > 1 and not skip_cc:
                nc.gpsimd.collective_compute(
                    "AllGather", OP.bypass,
                    replica_groups=[list(range(C))],
                    ins=[d_tab2l[:, :]],
                    outs=[d_tab2g[:, :]],
                )

            # ---------------- layer-2 edges + output ----------------
            OUTW = cfg.OUT
            with tc.tile_pool(name="w2", bufs=2) as wpool, \
                 tc.tile_pool(name="g2", bufs=2) as gpool, \
                 tc.tile_pool(name="e2ps", bufs=2, space="PSUM") as e2ps, \
                 tc.tile_pool(name="tps2", bufs=2, space="PSUM") as tps2:
                for b, ps, inv in edge_phase(2, wpool, gpool, e2ps, tps2):
                    inv8 = wpool.tile([P, 8], F32, tag="inv8")
                    nc.scalar.mul(inv8[:], inv[:], 0.125)
                    w_ = wpool.tile([P, OUTW, 8], F32, tag="w_")
                    nc.vector.tensor_tensor(
                        out=w_[:],
                        in0=ps[:, 0:F2].rearrange("p (d h) -> p d h", h=8),
                        in1=inv8[:].rearrange("p (one h) -> p one h", one=1)
                            .to_broadcast([P, OUTW, 8]),
                        op=OP.mult)
                    s1 = wpool.tile([P, OUTW, 4], F32, tag="s1")
                    nc.vector.tensor_add(s1[:], w_[:, :, 0:4], w_[:, :, 4:8])
                    s2 = wpool.tile([P, OUTW, 2], F32, tag="s2")
                    nc.vector.tensor_add(s2[:], s1[:, :, 0:2], s1[:, :, 2:4])
                    ob = wpool.tile([P, OUTW], F32, tag="ob")
                    nc.vector.tensor_add(
                        ob[:].rearrange("p (d one) -> p d one", one=1),
                        s2[:, :, 0:1], s2[:, :, 1:2])
                    nc.vector.tensor_add(outacc[:, b, :], ob[:], b2mean[:])
                NBF = cfg.Nloc // P                 # full output blocks
                nc.sync.dma_start(
                    d_out[0:NBF * P, :].rearrange("(t p) o -> p t o", p=P),
                    outacc[:, 0:NBF, :])
                rem = cfg.Nloc - NBF * P
                if rem > 0:
                    nc.sync.dma_start(d_out[NBF * P:cfg.Nloc, :],
                                      outacc[0:rem, NBF, :])

            if dbg:
                for nm, src_t in [("dbg_tab1l", d_tab1l), ("dbg_tab2l",
                                                           d_tab2l),
                                  ("dbg_tab1g", d_tab1g),
                                  ("dbg_tab2g", d_tab2g)]:
                    dd = nc.dram_tensor(nm, list(src_t.shape), BF16,
                                        kind="ExternalOutput")
                    sl = tuple(slice(None) for _ in src_t.shape)
                    nc.sync.dma_start(dd[sl], src_t[sl])

    nc.compile()
    return nc


# ----------------------------------------------------------------------------
_CACHE = {}


def get_built(src, dst, C=8, cfg=None):
    key = (hash(src.tobytes()), hash(dst.tobytes()), C)
    if key not in _CACHE:
        if cfg is None:
            cfg = GATCfg(C=C)
        idx = prep_indices(src, dst, cfg)
        nc = build_module(cfg)
        _CACHE[key] = (cfg, idx, nc)
    return _CACHE[key]


_EXECC = {}


def _get_exec(key, nc, n_cores):
    """Persistent jit(shard_map(bass_exec)) so repeated kernel() calls skip
    retracing/recompiling."""
    if key in _EXECC:
        return _EXECC[key]
    import jax
    from jax.experimental.shard_map import shard_map
    from jax.sharding import Mesh, NamedSharding, PartitionSpec
    from concourse import bass2jax
    bass2jax.install_neuronx_cc_hook()
    partition_name = (nc.partition_id_tensor.name
                      if nc.partition_id_tensor else None)
    in_names, out_names, out_avals, zero_shapes = [], [], [], []
    for alloc in nc.m.functions[0].allocations:
        if not isinstance(alloc, mybir.MemoryLocationSet):
            continue
        name = alloc.memorylocations[0].name
        if alloc.kind == "ExternalInput":
            if name != partition_name:
                in_names.append(name)
        elif alloc.kind == "ExternalOutput":
            out_names.append(name)
            shape = tuple(alloc.tensor_shape)
            dtype = mybir.dt.np(alloc.dtype)
            out_avals.append(jax.core.ShapedArray(shape, dtype))
            zero_shapes.append((shape, dtype))
    n_params = len(in_names)
    in_names_all = list(in_names) + out_names + (
        [partition_name] if partition_name else [])

    def _body(*args):
        ops = list(args)
        if partition_name:
            ops.append(bass2jax.partition_id_tensor())
        outs = bass2jax._bass_exec_p.bind(
            *ops, out_avals=tuple(out_avals), in_names=tuple(in_names_all),
            out_names=tuple(out_names), lowering_input_output_aliases=(),
            sim_require_finite=True, sim_require_nnan=True, nc=nc)
        return tuple(outs)

    devices = jax.devices()[:n_cores]
    mesh = Mesh(np.asarray(devices), ("core",))
    nout = len(out_names)
    f = jax.jit(shard_map(
        _body, mesh=mesh,
        in_specs=(PartitionSpec("core"),) * (n_params + nout),
        out_specs=(PartitionSpec("core"),) * nout, check_rep=False),
        keep_unused=True)
    sh = NamedSharding(mesh, PartitionSpec("core"))
    ent = dict(f=f, in_names=in_names, out_names=out_names,
               zero_shapes=zero_shapes, sh=sh, argcache=None)
    _EXECC[key] = ent
    return ent


def kernel(**inputs) -> np.ndarray:
    import jax
    src = np.asarray(inputs["src"], np.int32)
    dst = np.asarray(inputs["dst"], np.int32)
    x = np.asarray(inputs["x"])
    base = GATCfg(N=int(x.shape[0]), C=8, IN=int(x.shape[1]))
    cfg, idx, nc = get_built(src, dst, C=8, cfg=base)
    in_maps = host_inputs(inputs, cfg, idx)
    key = (hash(src.tobytes()), hash(dst.tobytes()), cfg.C)
    ent = _get_exec(key, nc, cfg.C)
    C = cfg.C
    concat_in = [np.ascontiguousarray(
        np.concatenate([in_maps[c][nm] for c in range(C)], axis=0))
        for nm in ent["in_names"]]
    hashes = tuple(hash(a.tobytes()) for a in concat_in)
    if ent["argcache"] is None or ent["argcache"][0] != hashes:
        zeros = [np.zeros((C * sh0[0], *sh0[1:]), dt)
                 for sh0, dt in ent["zero_shapes"]]
        args = [jax.device_put(a, ent["sh"]) for a in concat_in + zeros]
        ent["argcache"] = (hashes, args)
    args = ent["argcache"][1]
    outs = ent["f"](*args)
    jax.block_until_ready(outs)
    oi = ent["out_names"].index("out")
    out = np.asarray(outs[oi]).reshape(C, cfg.Nloc, cfg.OUT)
    return out.reshape(-1, cfg.OUT)[:cfg.N].astype(np.float32)


# revision 43
# speedup vs baseline: 1.0537x; 1.0537x over previous
"""2-layer GAT (graph attention) Bass/Tile kernel for Trainium2, 8-core SPMD.

Sharding: nodes partitioned contiguously across cores (rank-major ids
id = r*Nlp + local); edges assigned to the core owning their dst, sorted by
dst, grouped into 128-dst blocks with uniform slot padding so all cores share
one SPMD module.

Per core: build feature-table rows [feat bf16 | el bf16 | pad] for the LOCAL
nodes only (one matmul per 128-node block against a host-prebuilt rhs that
also produces the attention el/er columns), then AllGather the table so every
core holds all N rows.  Edge phase per 128-dst block: two dma_gathers (lo/hi
int16 index halves) fetch src rows; one-hot dst masks are built in a single
DVE op per block and drive PE matmuls that broadcast er to edges and
scatter-add the exp-weighted messages + exp-sums into PSUM (softmax
max-subtraction is skipped; |e| is O(1)).  The layer-2 table row (h@W2 and
its el/er columns) is computed inline right after layer-1 normalization+ELU,
so hidden states never round-trip HBM; a second AllGather publishes tab2.
er columns for local dst nodes stay resident in SBUF.  Outputs accumulate in
SBUF and are written once.
"""

import os

import numpy as np

import concourse.bacc as bacc
import concourse.bass as bass
import concourse.mybir as mybir
import concourse.tile as tile
from concourse.masks import make_identity

F32 = mybir.dt.float32
BF16 = mybir.dt.bfloat16
I32 = mybir.dt.int32
I16 = mybir.dt.int16
I8 = mybir.dt.int8
AF = mybir.ActivationFunctionType
OP = mybir.AluOpType

P = 128
NPBF = None  # numpy bfloat16 dtype, resolved lazily


def _npbf():
    global NPBF
    if NPBF is None:
        NPBF = mybir.dt.np(BF16)
    return NPBF


class GATCfg:
    def __init__(self, N=50000, C=8, IN=128, HID=32, HEADS=8, OUT=16, NEG=0.2):
        self.N, self.C, self.IN = N, C, IN
        self.HID, self.HEADS, self.OUT, self.NEG = HID, HEADS, OUT, NEG
        self.F1 = HEADS * HID
        self.F2 = HEADS * OUT
        self.Nloc = (N + C - 1) // C
        self.NB = (self.Nloc + P - 1) // P
        self.Nlp = self.NB * P
        self.NP2 = C * self.Nlp            # rank-major padded id space
        # int16 gather index split: lo ids < HALF, hi ids in [HALF, NP2)
        h = (self.NP2 // 2 + P - 1) // P * P
        self.HALF = min(32768, max(h, self.NP2 - 32768))
        assert self.HALF <= 32768 and self.NP2 - self.HALF <= 32768
        self.SLO = 0
        self.SHI = 0
        # table rows (bf16 elems), 256B-multiple for dma_gather
        self.ROW1 = ((self.F1 + 8 + 127) // 128) * 128   # 384 -> 768B
        self.ROW2 = ((self.F2 + 8 + 127) // 128) * 128   # 256 -> 512B
        self.GRP = 6   # metadata load group (even: gathers pair 2 blocks)

    @property
    def NCt(self):
        return (self.SLO + self.SHI) // P


def _wrap16(vals_slots, NB, S):
    """[NB, S] slot-ordered ints -> [128, NB*S//16] 16-wrapped, replicated 8x."""
    a = vals_slots.reshape(NB, S // 16, 16)
    out = np.zeros((128, NB * (S // 16)), np.int16)
    for b in range(NB):
        blkcols = a[b].T.astype(np.int16)          # [16, S//16]
        for r in range(8):
            out[r * 16:(r + 1) * 16,
                b * (S // 16):(b + 1) * (S // 16)] = blkcols
    return out


def prep_indices(src, dst, cfg):
    """Host index-only preprocessing (single shared id space, rank-major)."""
    C, Nloc, NB = cfg.C, cfg.Nloc, cfg.NB
    src = np.asarray(src).astype(np.int64)
    dst = np.asarray(dst).astype(np.int64)
    core = dst // Nloc
    dloc = dst - core * Nloc
    blk = dloc // P
    dblk = dloc - blk * P
    key = core * NB + blk
    order = np.argsort(key, kind="stable")
    counts = np.bincount(key, minlength=C * NB)
    starts = np.zeros(C * NB + 1, np.int64)
    np.cumsum(counts, out=starts[1:])

    r_of = src // Nloc
    ids = r_of * cfg.Nlp + (src - r_of * Nloc)     # rank-major id
    HALFc = cfg.HALF

    lo_max = hi_max = 0
    for k in range(C * NB):
        e = order[starts[k]:starts[k + 1]]
        n_lo = int((ids[e] < HALFc).sum())
        lo_max = max(lo_max, n_lo)
        hi_max = max(hi_max, e.size - n_lo)
    SLO = ((max(lo_max, 1) + P - 1) // P) * P
    SHI = ((max(hi_max, 1) + P - 1) // P) * P
    cfg.SLO, cfg.SHI = SLO, SHI
    S = SLO + SHI
    NCt = S // P

    idxlo = np.zeros((C, NB, SLO), np.int64)
    idxhi = np.zeros((C, NB, max(SHI, 16)), np.int64)
    dstb = np.full((C, P, NB * NCt), -1.0, np.float32)
    dbf = np.full((C, NB * NCt * P), -1, np.int8)
    for c in range(C):
        for b in range(NB):
            k = c * NB + b
            e = order[starts[k]:starts[k + 1]]
            v = ids[e]
            m = v < HALFc
            elo, ehi = e[m], e[~m]
            idxlo[c, b, :elo.size] = v[m]
            if ehi.size:
                idxhi[c, b, :ehi.size] = v[~m] - HALFc
            sl = np.concatenate([np.arange(elo.size),
                                 SLO + np.arange(ehi.size)])
            ee = np.concatenate([elo, ehi])
            dstb[c, sl % P, b * NCt + sl // P] = dblk[ee]
            dbf[c, b * NCt * P + sl] = dblk[ee]
    wlo = np.stack([_wrap16(idxlo[c], NB, SLO) for c in range(C)])
    whi = np.stack([_wrap16(idxhi[c], NB, max(SHI, 16)) for c in range(C)])
    return {"idxlo": wlo, "idxhi": whi, "dstb": dstb, "dbb": dbf}


def _alr_mat(al, ar, F, H, D):
    m = np.zeros((F, 16), np.float32)
    for h in range(H):
        m[h * D:(h + 1) * D, h] = al[h]
        m[h * D:(h + 1) * D, 8 + h] = ar[h]
    return m


def _dmaj(a, heads):
    """Permute the last axis from head-major (h*D+d) to d-major (d*H+h)."""
    D = a.shape[-1] // heads
    return np.ascontiguousarray(
        a.reshape(*a.shape[:-1], heads, D).swapaxes(-1, -2)
        .reshape(*a.shape[:-1], heads * D))


def host_inputs(inputs, cfg, idx):
    nbf = _npbf()
    H = cfg.HEADS
    x = np.asarray(inputs["x"], np.float32)
    W1 = np.asarray(inputs["W1"], np.float32)
    W2 = np.asarray(inputs["W2"], np.float32)
    alr1 = _alr_mat(np.asarray(inputs["al1"], np.float32),
                    np.asarray(inputs["ar1"], np.float32),
                    cfg.F1, cfg.HEADS, cfg.HID)
    alr2 = _alr_mat(np.asarray(inputs["al2"], np.float32),
                    np.asarray(inputs["ar2"], np.float32),
                    cfg.F2, cfg.HEADS, cfg.OUT)
    # feature columns stored d-major (d*H+h) so the per-head exp broadcast in
    # the edge phase has a packed innermost dim (2x DVE mode); el/er columns
    # stay head-indexed.
    rhs1 = np.concatenate([_dmaj(W1, H), W1 @ alr1], axis=1).astype(nbf)
    rhs2 = np.concatenate([_dmaj(W2, H), W2 @ alr2], axis=1)       # [256,144]
    # rows of rhs2 live in feat1 space -> permute to d-major to match hb
    rhs2 = np.ascontiguousarray(
        rhs2.reshape(H, cfg.HID, cfg.F2 + 16).swapaxes(0, 1)
        .reshape(cfg.F1, cfg.F2 + 16)).astype(nbf)
    b1row = np.broadcast_to(_dmaj(np.asarray(inputs["b1"], np.float32)
                                  .reshape(1, cfg.F1), H),
                            (P, cfg.F1)).astype(nbf)
    b2m = np.asarray(inputs["b2"], np.float32).reshape(
        cfg.HEADS, cfg.OUT).mean(axis=0)
    b2mean = np.broadcast_to(b2m.reshape(1, cfg.OUT), (P, cfg.OUT)).copy()

    xT = np.ascontiguousarray(x.T)                                  # [IN, N]
    in_maps = []
    for c in range(cfg.C):
        xTl = np.zeros((cfg.IN, cfg.Nlp), nbf)
        lo = c * cfg.Nloc
        hi = min(cfg.N, lo + cfg.Nloc)
        xTl[:, :hi - lo] = xT[:, lo:hi].astype(nbf)
        in_maps.append({
            "xTl": xTl, "RHS1": rhs1, "RHS2": rhs2,
            "B1R": b1row, "B2M": b2mean,
            "idxlo": idx["idxlo"][c], "idxhi": idx["idxhi"][c],
            "dstb": idx["dstb"][c].astype(nbf),
            "dbb": np.ascontiguousarray(
                np.broadcast_to(idx["dbb"][c][None, :],
                                (P, idx["dbb"].shape[1]))),
        })
    return in_maps


def build_module(cfg, dbg=False, skip_cc=False):
    nc = bacc.Bacc("TRN2", target_bir_lowering=False, debug=False,
                   num_devices=cfg.C, dynamic_dma_scratch_size=24576)
    C, NB, Nlp = cfg.C, cfg.NB, cfg.Nlp
    F1, F2, ROW1, ROW2 = cfg.F1, cfg.F2, cfg.ROW1, cfg.ROW2
    SLO, SHI, NCt = cfg.SLO, cfg.SHI, cfg.NCt
    NLOC, NHIC = SLO // P, SHI // P
    G = cfg.GRP
    NG = (NB + G - 1) // G
    CL = SLO // 16
    CH = max(SHI, 16) // 16

    d_xTl = nc.dram_tensor("xTl", [cfg.IN, Nlp], BF16, kind="ExternalInput")
    d_rhs1 = nc.dram_tensor("RHS1", [cfg.IN, F1 + 16], BF16,
                            kind="ExternalInput")
    d_rhs2 = nc.dram_tensor("RHS2", [F1, F2 + 16], BF16,
                            kind="ExternalInput")
    d_b1r = nc.dram_tensor("B1R", [P, F1], BF16, kind="ExternalInput")
    d_b2m = nc.dram_tensor("B2M", [P, cfg.OUT], F32, kind="ExternalInput")
    d_il = nc.dram_tensor("idxlo", [P, NB * CL], I16, kind="ExternalInput")
    d_ih = nc.dram_tensor("idxhi", [P, NB * CH], I16, kind="ExternalInput")
    d_db = nc.dram_tensor("dstb", [P, NB * NCt], BF16, kind="ExternalInput")
    d_dbb = nc.dram_tensor("dbb", [P, NB * NCt * P], I8, kind="ExternalInput")
    d_out = nc.dram_tensor("out", [cfg.Nloc, cfg.OUT], F32,
                           kind="ExternalOutput")

    # NOTE: dma_gather cannot read Shared-space tensors (neuronxcc
    # DataLocalityOpt asserts), so the AllGather outputs stay Local.
    shared = os.environ.get("GAT_CC_SPACE", "Local")
    d_tab1l = nc.dram_tensor("tab1l", [Nlp, ROW1], BF16, kind="Internal")
    d_tab2l = nc.dram_tensor("tab2l", [Nlp, ROW2], BF16, kind="Internal")
    d_tab1g = nc.dram_tensor("tab1g", [cfg.NP2, ROW1], BF16, kind="Internal",
                             addr_space=shared)
    d_tab2g = nc.dram_tensor("tab2g", [cfg.NP2, ROW2], BF16, kind="Internal",
                             addr_space=shared)

    with tile.TileContext(nc) as tc:
        with (
            tc.tile_pool(name="const", bufs=1) as cpool,
            tc.tile_pool(name="meta", bufs=2) as mpool,
        ):
            # ---------------- constants ----------------
            iota_i = cpool.tile([P, P], I32)
            nc.gpsimd.iota(iota_i[:], pattern=[[1, P]], base=0,
                           channel_multiplier=0)
            iota_p = cpool.tile([P, P], BF16)
            nc.vector.tensor_copy(iota_p[:], iota_i[:])
            # [P, j, c] iota (value j, constant along c): the oh build's db
            # broadcast then has a packed innermost dim -> 2x DVE mode
            iotaJc = cpool.tile([P, P, NCt], BF16)
            nc.vector.tensor_copy(
                iotaJc[:],
                iota_p[:].rearrange("p (j one) -> p j one", one=1)
                .to_broadcast([P, P, NCt]))
            iota_ci = cpool.tile([P, 1], I32)
            nc.gpsimd.iota(iota_ci[:], pattern=[[1, 1]], base=0,
                           channel_multiplier=1)
            iota_cf = cpool.tile([P, 1], F32)
            nc.vector.tensor_copy(iota_cf[:], iota_ci[:])
            ident = cpool.tile([P, P], BF16)
            make_identity(nc, ident[:])

            rhs1 = cpool.tile([P, F1 + 16], BF16)
            nc.sync.dma_start(rhs1[:], d_rhs1[:, :])
            rhs2 = cpool.tile([P, 2, F2 + 16], BF16)
            nc.sync.dma_start(rhs2[:, 0, :], d_rhs2[0:P, :])
            nc.sync.dma_start(rhs2[:, 1, :], d_rhs2[P:F1, :])
            b1row = cpool.tile([P, F1], BF16)
            nc.sync.dma_start(b1row[:], d_b1r[:, :])
            b2mean = cpool.tile([P, cfg.OUT], F32)
            nc.sync.dma_start(b2mean[:], d_b2m[:, :])

            er1_sb = cpool.tile([P, NB, 8], BF16)
            er2_sb = cpool.tile([P, NB, 8], BF16)
            tab2acc = cpool.tile([P, NB, F2 + 8], BF16)
            outacc = cpool.tile([P, NB, cfg.OUT], F32)

            # ---------------- layer-1 local table ----------------
            with tc.tile_pool(name="p1", bufs=1) as p1pool, \
                 tc.tile_pool(name="t1ps", bufs=3, space="PSUM") as t1ps:
                xTl_sb = p1pool.tile([P, Nlp], BF16)
                nc.sync.dma_start(xTl_sb[:], d_xTl[:, :])
                tab1acc = p1pool.tile([P, NB, F1 + 8], BF16)
                for t in range(NB):
                    ps = t1ps.tile([P, F1 + 16], F32, tag="tbl")
                    nc.tensor.matmul(ps[:], lhsT=xTl_sb[:, t * P:(t + 1) * P],
                                     rhs=rhs1[:], start=True, stop=True)
                    nc.scalar.copy(tab1acc[:, t, :], ps[:, 0:F1 + 8])
                    nc.vector.tensor_copy(er1_sb[:, t, :],
                                          ps[:, F1 + 8:F1 + 16])
                nc.sync.dma_start(
                    d_tab1l[:, 0:F1 + 8].rearrange("(t p) r -> p t r", p=P),
                    tab1acc[:])

            # ---------------- allgather tab1 ----------------
            if C > 1 and not skip_cc:
                nc.gpsimd.collective_compute(
                    "AllGather", OP.bypass,
                    replica_groups=[list(range(C))],
                    ins=[d_tab1l[:, :]],
                    outs=[d_tab1g[:, :]],
                )

            # ---------------- edge phase (shared) ----------------
            def edge_phase(layer, wpool, gpool, pspool, tps):
                F = F1 if layer == 1 else F2
                ROW = ROW1 if layer == 1 else ROW2
                if C > 1:
                    tab = d_tab1g if layer == 1 else d_tab2g
                else:
                    tab = d_tab1l if layer == 1 else d_tab2l
                er_sb = er1_sb if layer == 1 else er2_sb
                ablate = os.environ.get("GAT_ABLATE", "")
                pair = None
                for b in range(NB):
                    g = b // G
                    j = b % G
                    gn = min(G, NB - g * G)
                    if j == 0:
                        il_g = mpool.tile([P, G * CL], I16, tag="il")
                        nc.sync.dma_start(
                            il_g[:, 0:gn * CL],
                            d_il[:, g * G * CL:(g * G + gn) * CL])
                        ih_g = mpool.tile([P, G * CH], I16, tag="ih")
                        nc.sync.dma_start(
                            ih_g[:, 0:gn * CH],
                            d_ih[:, g * G * CH:(g * G + gn) * CH])
                        db_g = mpool.tile([P, G * NCt], BF16, tag="db")
                        nc.sync.dma_start(
                            db_g[:, 0:gn * NCt],
                            d_db[:, g * G * NCt:(g * G + gn) * NCt])
                        dbb_g = mpool.tile([P, G * NCt * P], I8, tag="dbb")
                        nc.sync.dma_start(
                            dbb_g[:, 0:gn * NCt * P],
                            d_dbb[:, g * G * NCt * P:(g * G + gn) * NCt * P])
                    PIECE = 768  # max idxs per dma_gather (packet cap)
                    if b % 2 == 0:
                        # gather a PAIR of blocks per tile so 768-idx pieces
                        # don't fragment at block boundaries
                        pn = min(2, NB - b)
                        G2lo = gpool.tile([P, 2 * NLOC, ROW], BF16,
                                          tag=f"Glo{layer}")
                        for s0 in ([] if "nogather" in ablate
                                   else range(0, pn * SLO, PIECE)):
                            n = min(PIECE, pn * SLO - s0)
                            nc.gpsimd.dma_gather(
                                out_ap=G2lo[:, s0 // P:(s0 + n) // P, :],
                                in_ap=tab[:, :],
                                idxs_ap=il_g[:, j * CL + s0 // 16:
                                             j * CL + (s0 + n) // 16],
                                num_idxs=n, num_idxs_reg=n, elem_size=ROW)
                        G2hi = gpool.tile([P, 2 * NHIC, ROW], BF16,
                                          tag=f"Ghi{layer}")
                        for s0 in ([] if "nogather" in ablate
                                   else range(0, pn * SHI, PIECE)):
                            n = min(PIECE, pn * SHI - s0)
                            nc.gpsimd.dma_gather(
                                out_ap=G2hi[:, s0 // P:(s0 + n) // P, :],
                                in_ap=tab[cfg.HALF:, :],
                                idxs_ap=ih_g[:, j * CH + s0 // 16:
                                             j * CH + (s0 + n) // 16],
                                num_idxs=n, num_idxs_reg=n, elem_size=ROW)
                        pair = (G2lo, G2hi)
                    G2lo, G2hi = pair
                    ho = (b % 2) * NLOC
                    hh = (b % 2) * NHIC

                    # one-hot masks, one DVE op each for the whole block
                    ohT = wpool.tile([P, NCt, P], BF16, tag="ohT")
                    nc.vector.tensor_scalar(
                        ohT[:], dbb_g[:, j * NCt * P:(j + 1) * NCt * P]
                        .rearrange("p (c j) -> p c j", c=NCt),
                        iota_cf[:, 0:1], None, op0=OP.is_equal)
                    oh = wpool.tile([P, P, NCt], BF16, tag="oh")
                    nc.vector.tensor_tensor(
                        out=oh[:], in0=iotaJc[:],
                        in1=db_g[:, j * NCt:(j + 1) * NCt]
                        .rearrange("p (one c) -> p one c", one=1)
                        .to_broadcast([P, P, NCt]),
                        op=OP.is_equal)

                    # er broadcast to edges: per-chunk PE matmul
                    erps = tps.tile([P, NCt, 8], F32, tag="erps")
                    for c in range(NCt):
                        nc.tensor.matmul(erps[:, c, :], lhsT=ohT[:, c, :],
                                         rhs=er_sb[:, b, :],
                                         start=True, stop=True)
                    e_all = wpool.tile([P, NCt, 8], F32, tag="e_all")
                    nc.vector.tensor_tensor(
                        out=e_all[:, 0:NLOC, :],
                        in0=G2lo[:, ho:ho + NLOC, F:F + 8],
                        in1=erps[:, 0:NLOC, :], op=OP.add)
                    nc.vector.tensor_tensor(
                        out=e_all[:, NLOC:NCt, :],
                        in0=G2hi[:, hh:hh + NHIC, F:F + 8],
                        in1=erps[:, NLOC:NCt, :], op=OP.add)
                    # leaky relu in one fused op: max(e, 0.2*e)
                    es_a = wpool.tile([P, NCt, 8], F32, tag="es_a")
                    nc.vector.scalar_tensor_tensor(
                        es_a[:], e_all[:], cfg.NEG, e_all[:],
                        op0=OP.mult, op1=OP.max)
                    MSG = gpool.tile([P, NCt, F + 8], BF16, tag=f"MSG{layer}")
                    nc.scalar.activation(MSG[:, :, F:F + 8], es_a[:], AF.Exp)

                    # feat cols are d-major: per-head exp broadcast has a
                    # packed innermost dim (2x DVE mode)
                    def _mult(mout, gin, exin, nchunk):
                        nc.vector.tensor_tensor(
                            out=mout.rearrange("p c (d h) -> p c d h", h=8),
                            in0=gin.rearrange("p c (d h) -> p c d h", h=8),
                            in1=exin.rearrange("p c (one h) -> p c one h",
                                               one=1)
                                .to_broadcast([P, nchunk, F // 8, 8]),
                            op=OP.mult)

                    _mult(MSG[:, 0:NLOC, 0:F], G2lo[:, ho:ho + NLOC, 0:F],
                          MSG[:, 0:NLOC, F:F + 8], NLOC)
                    _mult(MSG[:, NLOC:NCt, 0:F], G2hi[:, hh:hh + NHIC, 0:F],
                          MSG[:, NLOC:NCt, F:F + 8], NHIC)
                    ps = pspool.tile([P, F + 8], F32, tag="eps")
                    for c in range(NCt):
                        nc.tensor.matmul(ps[:], lhsT=oh[:, :, c],
                                         rhs=MSG[:, c, :],
                                         start=(c == 0), stop=(c == NCt - 1))

                    esum = wpool.tile([P, 8], F32, tag="esum")
                    nc.vector.tensor_scalar(esum[:], ps[:, F:F + 8], 1e-30,
                                            None, op0=OP.max)
                    inv = wpool.tile([P, 8], F32, tag="inv")
                    nc.vector.reciprocal(inv[:], esum[:])
                    yield b, ps, inv

            # ---------------- layer-1 edges + fused tab2 rows ----------------
            with tc.tile_pool(name="w1", bufs=2) as wpool, \
                 tc.tile_pool(name="g1", bufs=2) as gpool, \
                 tc.tile_pool(name="e1ps", bufs=2, space="PSUM") as e1ps, \
                 tc.tile_pool(name="tps", bufs=2, space="PSUM") as tps, \
                 tc.tile_pool(name="t2ps", bufs=2, space="PSUM") as t2ps:
                for b, ps, inv in edge_phase(1, wpool, gpool, e1ps, tps):
                    z = wpool.tile([P, F1], BF16, tag="z")
                    nc.vector.tensor_tensor(
                        out=z[:].rearrange("p (d h) -> p d h", h=8),
                        in0=ps[:, 0:F1].rearrange("p (d h) -> p d h", h=8),
                        in1=inv[:].rearrange("p (one h) -> p one h", one=1)
                            .to_broadcast([P, cfg.HID, 8]),
                        op=OP.mult)
                    nc.vector.tensor_add(z[:], z[:], b1row[:])
                    # elu(z) = relu(z) + (exp(min(z,0)) - 1)
                    zp = wpool.tile([P, F1], BF16, tag="zp")
                    nc.scalar.activation(zp[:], z[:], AF.Relu)
                    zm = wpool.tile([P, F1], BF16, tag="zm")
                    nc.vector.tensor_sub(zm[:], z[:], zp[:])
                    q_ = wpool.tile([P, F1], BF16, tag="q_")
                    nc.scalar.activation(q_[:], zm[:], AF.Exp)
                    hb = wpool.tile([P, F1], BF16, tag="hb")
                    nc.vector.scalar_tensor_tensor(
                        hb[:], q_[:], -1.0, zp[:], op0=OP.add, op1=OP.add)
                    # fused layer-2 table row: transpose h, matmul rhs2
                    ps2 = t2ps.tile([P, F2 + 16], F32, tag="ps2")
                    for q in range(2):
                        pst = t2ps.tile([P, P], BF16, tag="pst")
                        nc.tensor.transpose(pst[:], hb[:, q * P:(q + 1) * P],
                                            ident[:])
                        htp = wpool.tile([P, P], BF16, tag=f"htp{q}")
                        (nc.vector.tensor_copy if q == 0
                         else nc.scalar.copy)(htp[:], pst[:])
                        nc.tensor.matmul(ps2[:], lhsT=htp[:],
                                         rhs=rhs2[:, q, :],
                                         start=(q == 0), stop=(q == 1))
                    nc.scalar.copy(tab2acc[:, b, :], ps2[:, 0:F2 + 8])
                    nc.scalar.copy(er2_sb[:, b, :], ps2[:, F2 + 8:F2 + 16])
                nc.sync.dma_start(
                    d_tab2l[:, 0:F2 + 8].rearrange("(t p) r -> p t r", p=P),
                    tab2acc[:])

            # ---------------- allgather tab2 ----------------
            if C > 1 and not skip_cc:
                nc.gpsimd.collective_compute(
                    "AllGather", OP.bypass,
                    replica_groups=[list(range(C))],
                    ins=[d_tab2l[:, :]],
                    outs=[d_tab2g[:, :]],
                )

            # ---------------- layer-2 edges + output ----------------
            OUTW = cfg.OUT
            with tc.tile_pool(name="w2", bufs=2) as wpool, \
                 tc.tile_pool(name="g2", bufs=2) as gpool, \
                 tc.tile_pool(name="e2ps", bufs=2, space="PSUM") as e2ps, \
                 tc.tile_pool(name="tps2", bufs=2, space="PSUM") as tps2:
                for b, ps, inv in edge_phase(2, wpool, gpool, e2ps, tps2):
                    inv8 = wpool.tile([P, 8], F32, tag="inv8")
                    nc.scalar.mul(inv8[:], inv[:], 0.125)
                    w_ = wpool.tile([P, OUTW, 8], F32, tag="w_")
                    nc.vector.tensor_tensor(
                        out=w_[:],
                        in0=ps[:, 0:F2].rearrange("p (d h) -> p d h", h=8),
                        in1=inv8[:].rearrange("p (one h) -> p one h", one=1)
                            .to_broadcast([P, OUTW, 8]),
                        op=OP.mult)
                    s1 = wpool.tile([P, OUTW, 4], F32, tag="s1")
                    nc.vector.tensor_add(s1[:], w_[:, :, 0:4], w_[:, :, 4:8])
                    s2 = wpool.tile([P, OUTW, 2], F32, tag="s2")
                    nc.vector.tensor_add(s2[:], s1[:, :, 0:2], s1[:, :, 2:4])
                    ob = wpool.tile([P, OUTW], F32, tag="ob")
                    nc.vector.tensor_add(
                        ob[:].rearrange("p (d one) -> p d one", one=1),
                        s2[:, :, 0:1], s2[:, :, 1:2])
                    nc.vector.tensor_add(outacc[:, b, :], ob[:], b2mean[:])
                NBF = cfg.Nloc // P                 # full output blocks
                nc.sync.dma_start(
                    d_out[0:NBF * P, :].rearrange("(t p) o -> p t o", p=P),
                    outacc[:, 0:NBF, :])
                rem = cfg.Nloc - NBF * P
                if rem > 0:
                    nc.sync.dma_start(d_out[NBF * P:cfg.Nloc, :],
                                      outacc[0:rem, NBF, :])

            if dbg:
                for nm, src_t in [("dbg_tab1l", d_tab1l), ("dbg_tab2l",
                                                           d_tab2l),
                                  ("dbg_tab1g", d_tab1g),
                                  ("dbg_tab2g", d_tab2g)]:
                    dd = nc.dram_tensor(nm, list(src_t.shape), BF16,
                                        kind="ExternalOutput")
                    sl = tuple(slice(None) for _ in src_t.shape)
                    nc.sync.dma_start(dd[sl], src_t[sl])

    nc.compile()
    return nc


# ----------------------------------------------------------------------------
_CACHE = {}


def get_built(src, dst, C=8, cfg=None):
    key = (hash(src.tobytes()), hash(dst.tobytes()), C)
    if key not in _CACHE:
        if cfg is None:
            cfg = GATCfg(C=C)
        idx = prep_indices(src, dst, cfg)
        nc = build_module(cfg)
        _CACHE[key] = (cfg, idx, nc)
    return _CACHE[key]


_EXECC = {}


def _get_exec(key, nc, n_cores):
    """Persistent jit(shard_map(bass_exec)) so repeated kernel() calls skip
    retracing/recompiling."""
    if key in _EXECC:
        return _EXECC[key]
    import jax
    from jax.experimental.shard_map import shard_map
    from jax.sharding import Mesh, NamedSharding, PartitionSpec
    from concourse import bass2jax
    bass2jax.install_neuronx_cc_hook()
    partition_name = (nc.partition_id_tensor.name
                      if nc.partition_id_tensor else None)
    in_names, out_names, out_avals, zero_shapes = [], [], [], []
    for alloc in nc.m.functions[0].allocations:
        if not isinstance(alloc, mybir.MemoryLocationSet):
            continue
        name = alloc.memorylocations[0].name
        if alloc.kind == "ExternalInput":
            if name != partition_name:
                in_names.append(name)
        elif alloc.kind == "ExternalOutput":
            out_names.append(name)
            shape = tuple(alloc.tensor_shape)
            dtype = mybir.dt.np(alloc.dtype)
            out_avals.append(jax.core.ShapedArray(shape, dtype))
            zero_shapes.append((shape, dtype))
    n_params = len(in_names)
    in_names_all = list(in_names) + out_names + (
        [partition_name] if partition_name else [])

    def _body(*args):
        ops = list(args)
        if partition_name:
            ops.append(bass2jax.partition_id_tensor())
        outs = bass2jax._bass_exec_p.bind(
            *ops, out_avals=tuple(out_avals), in_names=tuple(in_names_all),
            out_names=tuple(out_names), lowering_input_output_aliases=(),
            sim_require_finite=True, sim_require_nnan=True, nc=nc)
        return tuple(outs)

    devices = jax.devices()[:n_cores]
    mesh = Mesh(np.asarray(devices), ("core",))
    nout = len(out_names)
    f = jax.jit(shard_map(
        _body, mesh=mesh,
        in_specs=(PartitionSpec("core"),) * (n_params + nout),
        out_specs=(PartitionSpec("core"),) * nout, check_rep=False),
        keep_unused=True)
    sh = NamedSharding(mesh, PartitionSpec("core"))
    ent = dict(f=f, in_names=in_names, out_names=out_names,
               zero_shapes=zero_shapes, sh=sh, argcache=None)
    _EXECC[key] = ent
    return ent


def kernel(**inputs) -> np.ndarray:
    import jax
    src = np.asarray(inputs["src"], np.int32)
    dst = np.asarray(inputs["dst"], np.int32)
    x = np.asarray(inputs["x"])
    base = GATCfg(N=int(x.shape[0]), C=8, IN=int(x.shape[1]))
    cfg, idx, nc = get_built(src, dst, C=8, cfg=base)
    in_maps = host_inputs(inputs, cfg, idx)
    key = (hash(src.tobytes()), hash(dst.tobytes()), cfg.C)
    ent = _get_exec(key, nc, cfg.C)
    C = cfg.C
    concat_in = [np.ascontiguousarray(
        np.concatenate([in_maps[c][nm] for c in range(C)], axis=0))
        for nm in ent["in_names"]]
    hashes = tuple(hash(a.tobytes()) for a in concat_in)
    if ent["argcache"] is None or ent["argcache"][0] != hashes:
        zeros = [np.zeros((C * sh0[0], *sh0[1:]), dt)
                 for sh0, dt in ent["zero_shapes"]]
        args = [jax.device_put(a, ent["sh"]) for a in concat_in + zeros]
        ent["argcache"] = (hashes, args)
    args = ent["argcache"][1]
    outs = ent["f"](*args)
    jax.block_until_ready(outs)
    oi = ent["out_names"].index("out")
    out = np.asarray(outs[oi]).reshape(C, cfg.Nloc, cfg.OUT)
    return out.reshape(-1, cfg.OUT)[:cfg.N].astype(np.float32)
